# revision 1
# baseline (speedup 1.0000x reference)
"""Trainium2 Bass kernel for nn_MultiHeadSelfAttention_67559835566279.

Module: x -> [sep_conv(q, stride1), sep_conv(kv, stride2)] -> 3-head attention
        -> output projection.  B=8 samples, data-parallel: one sample per core.

Per-core layout strategy (all "transposed" activations keep channels on SBUF
partitions so every matmul contracts along partitions):
  - host pre-pads/transposes x to xT_pad [C=192, 58*58] bf16 (zero border)
  - depthwise convs: per-tap free-dim-shifted views of xT_pad
      q branch: 9 diag-matmuls on PE (or DVE scalar_tensor_tensor path)
      kv branch: 9 diag-matmuls on PE with stride-2 views
  - pointwise convs as matmuls contracting C (chunks 128+64/65)
      qT [192, 3136], kT [192, 784] transposed; V [784, 192] natural
  - attention per head: S^T[j,i] = kT.T-slices @ qT, exp on ACT (PSUM->SBUF
    bf16), PV accumulates o^T[65, i] with a ones-column in V giving softmax
    denominators for free in row 0
  - normalization: DVE reciprocal of the sums row, DMA partition-broadcast,
    fused into the PSUM evacuation multiply
  - projection: o^T tiles as stationary, ones-row gives +out_b; output lands
    in natural [n, 192] layout, contiguous DMA out
"""

import numpy as np
import ml_dtypes

import concourse.bass as bass
import concourse.tile as tile
from concourse import bacc
from concourse import mybir
from contextlib import ExitStack

F32 = mybir.dt.float32
BF16 = mybir.dt.bfloat16
AOP = mybir.AluOpType
AF = mybir.ActivationFunctionType

B = 8
C = 192
H = W = 56
HP = 58                      # padded spatial
NPAD = HP * HP               # 3364
N = H * W                    # 3136
NKV = 28 * 28                # 784
HEADS = 3
DH = 64
EPS = 1e-5
SCALE = np.float32(64.0) ** np.float32(-0.5)

CC = [(0, 128), (128, 64)]   # channel chunks (offset, size)
DC = [(0, 128), (128, 64)]   # inner-dim chunks
TAPS = [(kh, kw) for kh in range(3) for kw in range(3)]

# Engine/buffering knobs (tuned against the TimelineSim cost model):
#  dwq/dve_taps: first 3 depthwise-q taps run as a DVE scalar_tensor_tensor
#    chain, remaining 6 as PE diag-matmuls accumulated in PSUM
#  conv_evac: conv PSUM evacuations on the (otherwise idle) scalar engine
#  proj=tail: output projection as a separate tail phase with deep pools
KNOBS = {"dwq": "pe", "phases": "all", "s_bufs": 2, "o_bufs": 2, "e_bufs": 3, "proj": "tail", "osz": 1024, "proj_evac": "act", "osb_bufs": 8, "pr_bufs": 4, "conv_evac": "act", "dve_taps": 3, "dwkv": "pe", "dve_exp": 0, "gp_taps": 0, "dw_bufs": 4, "pw_bufs": 4}

# exp(s) ~ ((s+EXP_A) * (EXP_C3*(s+EXP_HD)^2 + EXP_ADD))^2 on s in [-0.9, 0.9]
# (squared-cubic fit of e^{s/2}; max rel err 4.3e-4)
EXP_A = 3.2753946247435612
EXP_HD = 1.443304809557661
EXP_C3 = 0.020622990534789772
EXP_ADD = 0.2622859034294811
DVE_EXP_JS = {0: (), 1: (3,), 2: (2, 5), 3: (1, 3, 5)}

I_CHUNKS = [(3072, 64), (0, 1024), (1024, 1024), (2048, 1024)]
J_SZ = 112                   # 784 = 7 * 112
N_TILES = [(i * 128, min(128, N - i * 128)) for i in range((N + 127) // 128)]


def _as_f32(a):
    return np.ascontiguousarray(np.asarray(a, dtype=np.float32))


def build_nc(debug_taps=False, repeat=1, **knobs):
    KNOBS.update(knobs)
    nc = bacc.Bacc("TRN2", target_bir_lowering=False, debug=False, num_devices=B)

    din = {}
    def dram_in(name, shape, dtype):
        din[name] = nc.dram_tensor(name, shape, dtype, kind="ExternalInput").ap()
        return din[name]

    xtp = dram_in("xtp", [C, NPAD], BF16)
    dram_in("dwq1", [128, 9], F32)        # per-partition tap scalars (DVE path)
    dram_in("dwq2", [64, 9], F32)
    dram_in("dwkv1c", [128, 9], F32)
    dram_in("dwkv2c", [64, 9], F32)
    dram_in("qd1", [128, 9 * 128], BF16)  # diag tap matrices (PE path)
    dram_in("qd2", [64, 9 * 64], BF16)
    dram_in("kvd1", [128, 9 * 128], BF16)
    dram_in("kvd2", [64, 9 * 64], BF16)
    dram_in("pwq1", [128, 192], BF16)
    dram_in("pwq2", [64, 192], BF16)
    dram_in("pwkv1", [128, 384], BF16)
    dram_in("pwkv2", [65, 384], BF16)     # row 64: [0:192]=0, [192:384]=b_v
    dram_in("waug1", [128, 192], BF16)
    dram_in("waug2", [65, 192], BF16)     # row 64 = out_b
    dram_in("bias1", [128, 2], F32)       # col0=b_q col1=b_k
    dram_in("bias2", [64, 2], F32)

    out = nc.dram_tensor("out", [N, C], F32, kind="ExternalOutput").ap()
    dbg = {}
    if debug_taps:
        for nm, shp in [("d_qt", [C, N]), ("d_kt", [C, NKV]), ("d_v", [NKV, C]),
                        ("d_dwq", [C, N]), ("d_dwkv", [C, NKV])]:
            dbg[nm] = nc.dram_tensor(nm, shp, F32, kind="ExternalOutput").ap()

    with tile.TileContext(nc) as tc:
        for rep in range(repeat):
            with ExitStack() as ctx:
                _emit(ctx, tc, din, out, dbg, suffix=f"_r{rep}" if repeat > 1 else "")
    nc.compile()
    return nc


def _emit(ctx, tc, din, out, dbg, suffix=""):
    nc = tc.nc
    def _nm(s):
        return s + suffix

    def conv_evac(dst, src_ps, bias_ap=None, accum=False):
        if accum:
            nc.vector.tensor_tensor(out=dst, in0=src_ps, in1=dst, op=AOP.add)
        elif KNOBS["conv_evac"] == "act":
            if bias_ap is None:
                nc.scalar.copy(dst, src_ps)
            else:
                nc.scalar.activation(out=dst, in_=src_ps,
                                     func=AF.Identity, bias=bias_ap, scale=1.0)
        else:
            if bias_ap is None:
                nc.vector.tensor_copy(dst, src_ps)
            else:
                nc.vector.tensor_scalar(out=dst, in0=src_ps, scalar1=bias_ap,
                                        scalar2=None, op0=AOP.add)
    consts = ctx.enter_context(tc.tile_pool(name="consts", bufs=1))
    acts = ctx.enter_context(tc.tile_pool(name="acts", bufs=1))

    # ---- static loads -------------------------------------------------
    def load(name, shape=None, dtype=None):
        src = din[name]
        t = consts.tile(list(shape or src.shape), dtype or src.dtype, tag=name)
        nc.sync.dma_start(out=t[:, :], in_=src[:, :])
        return t

    xtp1 = consts.tile([128, NPAD], BF16, tag="xtp1", name=_nm("xtp1"))
    xtp2 = consts.tile([64, NPAD], BF16, tag="xtp2", name=_nm("xtp2"))
    for f0, f1 in [(0, NPAD // 2), (NPAD // 2, NPAD)]:
        nc.sync.dma_start(out=xtp1[:, f0:f1], in_=din["xtp"][0:128, f0:f1])
        nc.sync.dma_start(out=xtp2[:, f0:f1], in_=din["xtp"][128:192, f0:f1])
    xv = [xtp1[:, :].rearrange("p (h w) -> p h w", h=HP, w=HP),
          xtp2[:, :].rearrange("p (h w) -> p h w", h=HP, w=HP)]

    kvd = [load("kvd1"), load("kvd2")]
    dwq_w = [load("dwq1"), load("dwq2")]
    dwkv_w = [load("dwkv1c"), load("dwkv2c")]
    pwkv = [load("pwkv1"), load("pwkv2")]
    qd = [load("qd1"), load("qd2")]
    pwq = [load("pwq1"), load("pwq2")]
    waug = [load("waug1"), load("waug2")]
    bias = [load("bias1"), load("bias2")]

    # ---- activations (persistent SBUF) --------------------------------
    dwq_sb = [acts.tile([128, N], BF16, tag="dwq1s", name=_nm("dwq1s")),
              acts.tile([64, N], BF16, tag="dwq2s", name=_nm("dwq2s"))]
    dwkv_sb = [acts.tile([128, NKV], BF16, tag="dwkv1s", name=_nm("dwkv1s")),
               acts.tile([65, NKV], BF16, tag="dwkv2s", name=_nm("dwkv2s"))]   # row 64 = ones
    qT = [acts.tile([128, N], BF16, tag="qt1", name=_nm("qt1")),
          acts.tile([64, N], BF16, tag="qt2", name=_nm("qt2"))]
    kT = [acts.tile([128, NKV], BF16, tag="kt1", name=_nm("kt1")),
          acts.tile([64, NKV], BF16, tag="kt2", name=_nm("kt2"))]
    vsb = [acts.tile([J_SZ, 3 * 65], BF16, tag=f"v{j}", name=_nm(f"v{j}")) for j in range(7)]
    oTA = acts.tile([128, N], BF16, tag="oTA", name=_nm("oTA"))
    oTB = acts.tile([65, N], BF16, tag="oTB", name=_nm("oTB"))              # row 64 = ones

    nc.gpsimd.memset(dwkv_sb[1][64:65, :], 1.0)
    nc.gpsimd.memset(oTB[64:65, :], 1.0)
    for j in range(7):
        nc.gpsimd.memset(vsb[j][:, :].rearrange("p (h d) -> p h d", h=3)[:, :, 64:65], 1.0)

    def head_rows(tiles, h):
        # rows h*64:(h+1)*64 across the [128]+[64/65] tile pair
        return tiles[0][0:64] if h == 0 else (tiles[0][64:128] if h == 1 else tiles[1][0:64])

    # =========== phase B: convolutions =================================
    with ExitStack() as pctx:
        dw_ps = pctx.enter_context(tc.tile_pool(name="dw_ps", bufs=KNOBS["dw_bufs"], space="PSUM"))
        pw_ps = pctx.enter_context(tc.tile_pool(name="pw_ps", bufs=KNOBS["pw_bufs"], space="PSUM"))

        # ---- kv depthwise (stride-2 views) ----------------------------
        for ci, (c0, csz) in enumerate(CC):
            for j0, jn in [(0, 448), (448, 336)]:   # h' rows 0:16, 16:28
                h0 = (j0 // 28)
                if KNOBS["dwkv"] == "dve":
                    dst3 = dwkv_sb[ci][0:csz, j0:j0 + jn].rearrange(
                        "p (h w) -> p h w", h=jn // 28, w=28)
                    for t, (kh, kw) in enumerate(TAPS):
                        hs = 2 * h0 + kh + 1
                        ws = kw + 1
                        srcv = xv[ci][0:csz,
                                      hs: min(hs + 2 * (jn // 28), HP): 2,
                                      ws: min(ws + 56, HP): 2]
                        if t == 0:
                            nc.vector.tensor_scalar(
                                out=dst3, in0=srcv, scalar1=dwkv_w[ci][0:csz, 0:1],
                                scalar2=None, op0=AOP.mult)
                        else:
                            nc.vector.scalar_tensor_tensor(
                                out=dst3, in0=srcv, scalar=dwkv_w[ci][0:csz, t:t + 1],
                                in1=dst3, op0=AOP.mult, op1=AOP.add)
                    continue
                ps = dw_ps.tile([csz, 448], F32, tag="dw", name=_nm("dwkv_ps"))
                for t, (kh, kw) in enumerate(TAPS):
                    hs = 2 * h0 + kh + 1
                    ws = kw + 1
                    rhs = xv[ci][0:csz,
                                 hs: min(hs + 2 * (jn // 28), HP): 2,
                                 ws: min(ws + 56, HP): 2]
                    nc.tensor.matmul(out=ps[0:csz, 0:jn],
                                     lhsT=kvd[ci][0:csz, t * csz:(t + 1) * csz],
                                     rhs=rhs, start=(t == 0), stop=(t == 8))
                conv_evac(dwkv_sb[ci][0:csz, j0:j0 + jn], ps[0:csz, 0:jn])

        # ---- pointwise k (transposed out) -----------------------------
        for di, (d0, dsz) in enumerate(DC):
            for j0, jn in [(0, 448), (448, 336)]:
                ps = pw_ps.tile([dsz, 448], F32, tag="pw", name=_nm("pwk_ps"))
                for ci, (c0, csz) in enumerate(CC):
                    nc.tensor.matmul(out=ps[0:dsz, 0:jn],
                                     lhsT=pwkv[ci][0:csz, d0:d0 + dsz],
                                     rhs=dwkv_sb[ci][0:csz, j0:j0 + jn],
                                     start=(ci == 0), stop=(ci == 1))
                conv_evac(kT[di][0:dsz, j0:j0 + jn], ps[0:dsz, 0:jn],
                          bias_ap=bias[di][0:dsz, 1:2])

        # ---- pointwise v (natural out, ones-row K-augmentation) -------
        for j in range(7):
            ps = pw_ps.tile([J_SZ, 192], F32, tag="pw", name=_nm("pwv_ps"))
            js = slice(j * J_SZ, (j + 1) * J_SZ)
            nc.tensor.matmul(out=ps[:, :], lhsT=dwkv_sb[0][0:128, js],
                             rhs=pwkv[0][0:128, 192:384], start=True, stop=False)
            nc.tensor.matmul(out=ps[:, :], lhsT=dwkv_sb[1][0:65, js],
                             rhs=pwkv[1][0:65, 192:384], start=False, stop=True)
            conv_evac(
                vsb[j][:, :].rearrange("p (h d) -> p h d", h=3)[:, :, 0:64],
                ps[:, :].rearrange("p (h d) -> p h d", h=3))

        # ---- q depthwise + pointwise, streamed over 448-windows -------
        n_dve = KNOBS["dve_taps"] if KNOBS["dwq"] == "pe" else 9
        n_gp = KNOBS.get("gp_taps", 0) if KNOBS["dwq"] == "pe" else 0
        for w0 in range(0, N, 448):
            h0 = w0 // W
            for ci, (c0, csz) in enumerate(CC):
                dst3 = dwq_sb[ci][0:csz, w0:w0 + 448].rearrange(
                    "p (h w) -> p h w", h=8, w=56)
                for t in range(n_dve + n_gp):
                    eng = nc.vector if t < n_dve else nc.gpsimd
                    kh, kw = TAPS[t]
                    srcv = xv[ci][0:csz, h0 + kh:h0 + kh + 8, kw:kw + 56]
                    if t == 0:
                        eng.tensor_scalar(
                            out=dst3, in0=srcv, scalar1=dwq_w[ci][0:csz, 0:1],
                            scalar2=None, op0=AOP.mult)
                    else:
                        eng.scalar_tensor_tensor(
                            out=dst3, in0=srcv, scalar=dwq_w[ci][0:csz, t:t + 1],
                            in1=dst3, op0=AOP.mult, op1=AOP.add)
                if n_dve + n_gp < 9:
                    ps = dw_ps.tile([csz, 448], F32, tag="dw", name=_nm("dwq_ps"))
                    for t in range(n_dve + n_gp, 9):
                        kh, kw = TAPS[t]
                        rhs = xv[ci][0:csz, h0 + kh:h0 + kh + 8, kw:kw + 56]
                        nc.tensor.matmul(out=ps[0:csz, 0:448],
                                         lhsT=qd[ci][0:csz, t * csz:(t + 1) * csz],
                                         rhs=rhs, start=(t == n_dve + n_gp), stop=(t == 8))
                    conv_evac(dwq_sb[ci][0:csz, w0:w0 + 448], ps[0:csz, 0:448],
                              accum=(n_dve + n_gp > 0))
            for di, (d0, dsz) in enumerate(DC):
                ps = pw_ps.tile([dsz, 448], F32, tag="pw", name=_nm("pwq_ps"))
                for ci, (c0, csz) in enumerate(CC):
                    nc.tensor.matmul(out=ps[0:dsz, 0:448],
                                     lhsT=pwq[ci][0:csz, d0:d0 + dsz],
                                     rhs=dwq_sb[ci][0:csz, w0:w0 + 448],
                                     start=(ci == 0), stop=(ci == 1))
                conv_evac(qT[di][0:dsz, w0:w0 + 448], ps[0:dsz, 0:448],
                          bias_ap=bias[di][0:dsz, 0:1])

    if dbg:
        for ci, (c0, csz) in enumerate(CC):
            nc.gpsimd.dma_start(out=dbg["d_dwq"][c0:c0 + csz, :], in_=dwq_sb[ci][0:csz, :])
            nc.gpsimd.dma_start(out=dbg["d_dwkv"][c0:c0 + csz, :], in_=dwkv_sb[ci][0:csz, :])
            nc.gpsimd.dma_start(out=dbg["d_qt"][c0:c0 + csz, :], in_=qT[ci][0:csz, :])
            nc.gpsimd.dma_start(out=dbg["d_kt"][c0:c0 + csz, :], in_=kT[ci][0:csz, :])
        for j in range(7):
            nc.gpsimd.dma_start(
                out=dbg["d_v"][j * J_SZ:(j + 1) * J_SZ, :],
                in_=vsb[j][:, :].rearrange("p (h d) -> p h d", h=3)[:, :, 0:64])

    # =========== phase C: attention + projection (i-chunk outer) =======
    if KNOBS["phases"] == "conv":
        nc.gpsimd.dma_start(out=out[0:128, :], in_=qT[0][0:128, 0:192])
        return
    with ExitStack() as pctx:
        s_ps = pctx.enter_context(tc.tile_pool(name="s_ps", bufs=KNOBS["s_bufs"], space="PSUM"))
        o_ps = pctx.enter_context(tc.tile_pool(name="o_ps", bufs=KNOBS["o_bufs"], space="PSUM"))
        epool = pctx.enter_context(tc.tile_pool(name="expS", bufs=KNOBS["e_bufs"]))
        rpool = pctx.enter_context(tc.tile_pool(name="recip", bufs=2))
        rbpool = pctx.enter_context(tc.tile_pool(name="rbcast", bufs=2))
        rdpool = pctx.enter_context(tc.tile_pool(name="rdram", bufs=2, space="DRAM"))
        opool = pctx.enter_context(tc.tile_pool(name="outsb", bufs=KNOBS["osb_bufs"]))
        if KNOBS["osz"] == 512 and KNOBS["proj"] == "inline":
            pr_ps = pctx.enter_context(tc.tile_pool(name="pr_ps", bufs=2, space="PSUM"))
        elif KNOBS["proj"] == "inline_s":
            pr_ps = s_ps
        else:
            pr_ps = o_ps

        osz = KNOBS["osz"]
        for ic, (i0, isz) in enumerate(I_CHUNKS):
            for h in range(HEADS):
                q_h = head_rows(qT, h)
                k_h = head_rows(kT, h)
                o_tiles = []
                for g0 in range(0, isz, osz):
                    o_tiles.append((g0, min(osz, isz - g0),
                                    o_ps.tile([65, osz], F32, tag="o", name=_nm(f"o{ic}h{h}g{g0}"))))
                for j in range(7):
                    sps = s_ps.tile([J_SZ, 1024], F32, tag="s", name=_nm(f"s{ic}h{h}j{j}"))
                    for f0 in range(0, isz, 512):
                        fn = min(512, isz - f0)
                        nc.tensor.matmul(
                            out=sps[:, f0:f0 + fn],
                            lhsT=k_h[:, j * J_SZ:(j + 1) * J_SZ],
                            rhs=q_h[:, i0 + f0:i0 + f0 + fn],
                            start=True, stop=True)
                    es = epool.tile([J_SZ, 1024], BF16, tag="e", name=_nm(f"e{ic}h{h}j{j}"))
                    if j in DVE_EXP_JS[KNOBS["dve_exp"]]:
                        sc = epool.tile([J_SZ, 1024], BF16, tag="xsc", name=_nm(f"sc{ic}h{h}j{j}"))
                        f1 = epool.tile([J_SZ, 1024], BF16, tag="xf1", name=_nm(f"f1{ic}h{h}j{j}"))
                        g = epool.tile([J_SZ, 1024], BF16, tag="xg", name=_nm(f"g{ic}h{h}j{j}"))
                        sl = slice(0, isz)
                        nc.vector.tensor_scalar(out=sc[:, sl], in0=sps[:, sl],
                                                scalar1=0.0, scalar2=None, op0=AOP.add)
                        nc.vector.tensor_scalar(out=f1[:, sl], in0=sc[:, sl],
                                                scalar1=EXP_A, scalar2=None, op0=AOP.add)
                        nc.vector.tensor_scalar(out=g[:, sl], in0=sc[:, sl],
                                                scalar1=EXP_HD, scalar2=None, op0=AOP.add)
                        nc.vector.tensor_tensor(out=g[:, sl], in0=g[:, sl], in1=g[:, sl],
                                                op=AOP.mult)
                        nc.vector.tensor_scalar(out=g[:, sl], in0=g[:, sl],
                                                scalar1=EXP_C3, scalar2=EXP_ADD,
                                                op0=AOP.mult, op1=AOP.add)
                        nc.vector.tensor_tensor(out=g[:, sl], in0=f1[:, sl], in1=g[:, sl],
                                                op=AOP.mult)
                        nc.vector.tensor_tensor(out=es[:, sl], in0=g[:, sl], in1=g[:, sl],
                                                op=AOP.mult)
                    else:
                        nc.scalar.activation(out=es[:, 0:isz], in_=sps[:, 0:isz], func=AF.Exp)
                    for g0, gsz, ot in o_tiles:
                        for f0 in range(g0, g0 + gsz, 512):
                            fn = min(512, g0 + gsz - f0)
                            nc.tensor.matmul(
                                out=ot[:, f0 - g0:f0 - g0 + fn],
                                lhsT=vsb[j][:, h * 65:(h + 1) * 65],
                                rhs=es[:, f0:f0 + fn],
                                start=(j == 0), stop=(j == 6))
                dst = (oTA[0:64] if h == 0 else (oTA[64:128] if h == 1 else oTB[0:64]))
                for g0, gsz, ot in o_tiles:
                    rec = rpool.tile([1, 1024], F32, tag="r", name=_nm(f"r{ic}h{h}g{g0}"))
                    nc.vector.reciprocal(rec[:, 0:gsz], ot[64:65, 0:gsz])
                    rd = rdpool.tile([1, 1024], F32, tag="rd", name=_nm(f"rd{ic}h{h}g{g0}"))
                    nc.sync.dma_start(out=rd[:, 0:gsz], in_=rec[:, 0:gsz])
                    rb = rbpool.tile([64, 1024], F32, tag="rb", name=_nm(f"rb{ic}h{h}g{g0}"))
                    nc.gpsimd.dma_start(out=rb[:, 0:gsz],
                                        in_=rd[0:1, 0:gsz].to_broadcast((64, gsz)))
                    nc.vector.tensor_tensor(out=dst[:, i0 + g0:i0 + g0 + gsz],
                                            in0=ot[0:64, 0:gsz], in1=rb[:, 0:gsz],
                                            op=AOP.mult)
            if KNOBS["phases"] == "attn" or KNOBS["proj"] not in ("inline", "inline_s"):
                continue
            # ---- projection for the n-tiles covered by this i-chunk ----
            for n0 in range(i0, i0 + isz, 128):
                nsz = min(128, N - n0)
                ptag = "pr" if pr_ps.name == "pr_ps" else ("s" if pr_ps is s_ps else "o")
                ps = pr_ps.tile([nsz, 192], F32, tag=ptag, name=_nm(f"pr{n0}"))
                nc.tensor.matmul(out=ps[0:nsz, :], lhsT=oTA[:, n0:n0 + nsz],
                                 rhs=waug[0][:, :], start=True, stop=False)
                nc.tensor.matmul(out=ps[0:nsz, :], lhsT=oTB[0:65, n0:n0 + nsz],
                                 rhs=waug[1][0:65, :], start=False, stop=True)
                osb = opool.tile([nsz, 192], F32, tag="osb", name=_nm(f"osb{n0}"))
                if KNOBS["proj_evac"] == "act":
                    nc.scalar.copy(osb[0:nsz, :], ps[0:nsz, :])
                else:
                    nc.vector.tensor_copy(osb[0:nsz, :], ps[0:nsz, :])
                nc.sync.dma_start(out=out[n0:n0 + nsz, :], in_=osb[0:nsz, :])
        if KNOBS["phases"] == "attn":
            nc.gpsimd.dma_start(out=out[0:128, :], in_=oTA[0:128, 0:192])
            return



    if KNOBS["proj"] == "tail" and KNOBS["phases"] == "all":
        with ExitStack() as pctx:
            pr_ps2 = pctx.enter_context(tc.tile_pool(name="pr_ps2", bufs=KNOBS.get("pr_bufs", 6), space="PSUM"))
            opool2 = pctx.enter_context(tc.tile_pool(name="outsb2", bufs=KNOBS["osb_bufs"]))
            for n0, nsz in N_TILES:
                ps = pr_ps2.tile([nsz, 192], F32, tag="pr", name=_nm(f"pr{n0}"))
                nc.tensor.matmul(out=ps[0:nsz, :], lhsT=oTA[:, n0:n0 + nsz],
                                 rhs=waug[0][:, :], start=True, stop=False)
                nc.tensor.matmul(out=ps[0:nsz, :], lhsT=oTB[0:65, n0:n0 + nsz],
                                 rhs=waug[1][0:65, :], start=False, stop=True)
                osb = opool2.tile([nsz, 192], F32, tag="osb", name=_nm(f"osb{n0}"))
                if KNOBS["proj_evac"] == "act":
                    nc.scalar.copy(osb[0:nsz, :], ps[0:nsz, :])
                else:
                    nc.vector.tensor_copy(osb[0:nsz, :], ps[0:nsz, :])
                nc.sync.dma_start(out=out[n0:n0 + nsz, :], in_=osb[0:nsz, :])


# ======================= host-side preparation =========================

def prep_weights(inputs):
    """Fold BN, scale k by 1/sqrt(dh), build all packed weight arrays."""
    f = _as_f32
    bf = ml_dtypes.bfloat16
    qs = f(inputs["q_gamma"]) / np.sqrt(f(inputs["q_var"]) + EPS)
    qb = f(inputs["q_beta"]) - f(inputs["q_mean"]) * qs
    kvs = f(inputs["kv_gamma"]) / np.sqrt(f(inputs["kv_var"]) + EPS)
    kvb = f(inputs["kv_beta"]) - f(inputs["kv_mean"]) * kvs

    dwq = f(inputs["dw_q"])[:, :, 0, :] * qs          # [3,3,C]
    dwkv = f(inputs["dw_kv"])[:, :, 0, :] * kvs
    dwq_t = dwq.reshape(9, C).T.copy()                # [C, 9]
    dwkv_t = dwkv.reshape(9, C).T.copy()

    b_q = qb @ f(inputs["pw_q"])
    b_kv = kvb @ f(inputs["pw_kv"])
    pw_kv = f(inputs["pw_kv"]).copy()
    pw_kv[:, :192] *= SCALE
    b_k = b_kv[:192] * SCALE
    b_v = b_kv[192:]

    def diag_pack(wt, c0, csz):
        m = np.zeros((csz, 9 * csz), np.float32)
        for t in range(9):
            m[np.arange(csz), t * csz + np.arange(csz)] = wt[c0:c0 + csz, t]
        return m.astype(bf)

    pwkv2 = np.zeros((65, 384), np.float32)
    pwkv2[0:64] = pw_kv[128:192]
    pwkv2[64, 192:384] = b_v

    waug2 = np.zeros((65, 192), np.float32)
    waug2[0:64] = f(inputs["out_w"])[128:192]
    waug2[64] = f(inputs["out_b"])

    return {
        "dwq1": dwq_t[0:128].copy(), "dwq2": dwq_t[128:192].copy(),
        "dwkv1c": dwkv_t[0:128].copy(), "dwkv2c": dwkv_t[128:192].copy(),
        "qd1": diag_pack(dwq_t, 0, 128), "qd2": diag_pack(dwq_t, 128, 64),
        "kvd1": diag_pack(dwkv_t, 0, 128), "kvd2": diag_pack(dwkv_t, 128, 64),
        "pwq1": f(inputs["pw_q"])[0:128].astype(bf),
        "pwq2": f(inputs["pw_q"])[128:192].astype(bf),
        "pwkv1": pw_kv[0:128].astype(bf),
        "pwkv2": pwkv2.astype(bf),
        "waug1": f(inputs["out_w"])[0:128].astype(bf),
        "waug2": waug2.astype(bf),
        "bias1": np.stack([b_q[0:128], b_k[0:128]], axis=1).copy(),
        "bias2": np.stack([b_q[128:192], b_k[128:192]], axis=1).copy(),
    }


def prep_x(x):
    """[B,56,56,192] f32 -> list of per-sample padded transposed bf16."""
    bf = ml_dtypes.bfloat16
    x = _as_f32(x)
    xt = np.zeros((B, C, HP, HP), bf)
    xt[:, :, 1:57, 1:57] = x.transpose(0, 3, 1, 2).astype(bf)
    return [xt[b].reshape(C, NPAD) for b in range(B)]


_CACHED_NC = None


def _run(inputs, trace=False, **kwargs):
    global _CACHED_NC
    from concourse.bass_utils import run_bass_kernel_spmd

    if _CACHED_NC is None:
        _CACHED_NC = build_nc()
    nc = _CACHED_NC

    w = prep_weights(inputs)
    xs = prep_x(inputs["x"])
    in_maps = [dict(w, xtp=xs[b]) for b in range(B)]
    res = run_bass_kernel_spmd(nc, in_maps, list(range(B)), trace=trace, **kwargs)
    out = np.stack([np.asarray(res.results[b]["out"], np.float32) for b in range(B)])
    return out.reshape(B, H, W, C), res


def kernel(**inputs):
    return _run(inputs)[0]

